# revision 38
# baseline (speedup 1.0000x reference)
"""Sliding-window (banded) multi-head self-attention on 8 trn2 NeuronCores.

Sequence-parallel sharding: batch b, 2048 tokens -> 4 chunks of 512 queries;
core c handles batch c//4, chunk c%4.  Each core receives x^T for its 512
tokens plus a 128-token halo (zero-padded for chunk 0), computes
qkv projection + RoPE + banded attention (window 129) + out projection for
its rows, and returns [512, 2048].  No cross-core communication.

All matmul operands are bf16 (psum fp32): same 1 row/cycle PE rate as fp32r
but half the DMA / LDWEIGHTS traffic.  End-to-end numeric error vs the fp32
reference is ~5e-3, inside the 2e-2 gate.

Layout choices (all matmuls contract over the partition dim):
  - x^T resident in SBUF as [128, 16(e-chunk), 640(tok)]
  - Q^T/K^T per head feature-major [128(d), tok] straight out of PSUM;
    RoPE pairs de-interleaved host-side (d' = evens then odds) so
    rotate_half is a partition-half swap (SBUF->SBUF DMA).
  - V token-major [128(tok), d] (natural for PV lhsT).
  - attention in 128-query chunks: per (h, qc) one [128,256] PSUM tile
    holds scores^T for k-blocks qc (diag, upper-tri mask) and qc+1
    (lower-tri mask) side by side; one exp on ACT; one 0/1 mask multiply;
    2 PV + 2 replicated-ones rowsum matmuls accumulate into one poc bank;
    normalize with exp(-ln(rowsum)) on ACT.  Per query only 256 keys are
    scored (vs 384 with 256-query chunks) - 2/3 the attention PE work.
  - out projection accumulates 16 hd-chunks into per-head out_norm tiles
    (so its first matmuls wait only head 0's normalize); bias added on DVE;
    output stored bf16 (host casts back to fp32).
  - RoPE rotate-half: cos and (pre-swapped-table) sin muls both run on DVE
    straight from PSUM, the partition-half swap is an SBUF->SBUF DMA on the
    scalar queue, and the final add is back on DVE - no ACT copy in the
    chain, and no DMA-latency-coupled op ahead of the attention masks in
    the DVE stream.

Software pipeline with a 2-step skew (step s: project head s, attend head
s-2): every RoPE chain gets >1 step of slack, and the V-projection group 0
moves off the startup critical path (the first ~30us are DMA-bandwidth
bound).  Emission order keeps a couple microseconds of projection matmuls
in front of every dependent attention chain (exp->mask->PV, poc
recycling).  Startup DMA descriptor pushes are spread over the
sync/scalar/gpsimd queues.  Do NOT reorder the attend blocks or boost
their scheduler priority: the Tile scheduler is dataflow-driven and both
experiments regressed by 7-16us (measured).
"""

import math
import numpy as np
import ml_dtypes

import concourse.bass as bass
import concourse.tile as tile
from concourse import mybir
from concourse.bass_utils import run_bass_kernel_spmd
from concourse.vector_clock import ScopedClock, VectorClock


def _legalize_single_wait(nc):
    """This walrus build accepts only ONE sync-wait per lowered command
    ("Too many sync wait commands").  Move all but the last wait of every
    instruction onto single-wait NoOps prepended on the same engine: engines
    are in-order, so stalling on the NoOps is equivalent.  SP-issued DMAs are
    gated the same way (descriptor push happens in SP program order)."""
    nid = [0]
    for f in nc.m.functions:
        for blk in f.blocks:
            out = []
            changed = False
            for inst in blk.instructions:
                si = inst.sync_info
                waits = list(si.on_wait) if si and si.on_wait else []
                if len(waits) > 1:
                    changed = True
                    for w in waits[:-1]:
                        nop = mybir.InstNoOp(name=f"waitnop-{nid[0]}", ins=[], outs=[])
                        nid[0] += 1
                        nop.engine = inst.engine
                        nop.sync_info = mybir.SyncInfo(on_wait=[w], on_update=[])
                        out.append(nop)
                    inst.sync_info = mybir.SyncInfo(
                        on_wait=[waits[-1]], on_update=list(si.on_update or [])
                    )
                out.append(inst)
            if changed:
                blk.instructions = out
    return nc


def _install_drain_split_patch():
    """Split TileContext's closing drain into single-wait drains: walrus's
    CTRL_NO command rejects the catch-all drain ("Too many sync waits")."""
    if getattr(tile.TileContext, "_drain_split_patched", False):
        return

    def _patched(self, tick_clock, wait_clock):
        gvc = tick_clock.global_clock  # VectorClock over the 27 procs
        n = len(gvc)
        procs = [i for i in range(n) if gvc[i] > 0]
        for pi in procs:
            vc = VectorClock([gvc[i] if i == pi else 0 for i in range(n)])
            d = self.nc.sync.drain()
            wait_clock.add_sem_waits(d.ins, ScopedClock({None: vc}))
        self.nc.all_engine_barrier()
        assert self.sems is not None
        popped = self.nc._tile_sem_poison_stack.pop()
        assert popped is self._sem_poison
        self.nc.clear_and_free_semaphores(list(self.sems.allocated().values()))
        self.nc.all_engine_barrier()

    tile.TileContext._drain_and_barrier = _patched
    tile.TileContext._drain_split_patched = True


_install_drain_split_patch()

EMBED = 2048
HEADS = 16
HD = 128
WINDOW = 128
THETA = 10000.0
B = 2
L = 2048
S = 512            # queries per core
T = S + WINDOW     # k/v tokens per core (incl halo)
NCORES = 8
P = 128
F32 = mybir.dt.float32
BF16 = mybir.dt.bfloat16
HT1 = 384          # K-projection first token half (chunk-aligned)


def build_bass(legalize=True):
    nc = bass.Bass("TRN2", target_bir_lowering=False, debug=False)

    XT = nc.dram_tensor("XT", [P, EMBED // P, T], BF16, kind="ExternalInput")
    WQ = nc.dram_tensor("WQ", [HEADS, P, EMBED // P, HD], BF16, kind="ExternalInput")
    WK = nc.dram_tensor("WK", [HEADS, P, EMBED // P, HD], BF16, kind="ExternalInput")
    WV = nc.dram_tensor("WV", [4, P, EMBED // P, 512], BF16, kind="ExternalInput")
    WO = nc.dram_tensor("WO", [4, P, EMBED // P, 512], BF16, kind="ExternalInput")
    BOUT = nc.dram_tensor("BOUT", [P, EMBED], BF16, kind="ExternalInput")
    COSQ = nc.dram_tensor("COSQ", [P, S], BF16, kind="ExternalInput")
    SINQ = nc.dram_tensor("SINQ", [P, S], BF16, kind="ExternalInput")
    COSK = nc.dram_tensor("COSK", [P, T], BF16, kind="ExternalInput")
    SINK = nc.dram_tensor("SINK", [P, T], BF16, kind="ExternalInput")
    MASKS = nc.dram_tensor("MASKS", [2, P, 256], BF16, kind="ExternalInput")
    ONES = nc.dram_tensor("ONES", [P, P], BF16, kind="ExternalInput")
    OUT = nc.dram_tensor("OUT", [S, EMBED], BF16, kind="ExternalOutput")

    EC = EMBED // P  # 16 e-chunks
    GROUPS = 4       # head groups of 4 (for V projection at N=512)
    GH = HEADS // GROUPS

    with tile.TileContext(nc) as tc:
        with (
            tc.tile_pool(name="persist", bufs=1) as persist,
            tc.tile_pool(name="wbig", bufs=8) as wbig,
            tc.tile_pool(name="wbig4", bufs=8) as wbig4,
            tc.tile_pool(name="rope", bufs=3) as rope,
            tc.tile_pool(name="vsb", bufs=12) as vsb_pool,
            tc.tile_pool(name="attn", bufs=8) as attn_pool,
            tc.tile_pool(name="small", bufs=4) as small,
            tc.tile_pool(name="outsb", bufs=4) as outsb,
            tc.tile_pool(name="ps_qv", bufs=2, space="PSUM") as ps_qv,
            tc.tile_pool(name="ps_k", bufs=1, space="PSUM") as ps_k,
            tc.tile_pool(name="ps_sc", bufs=2, space="PSUM") as ps_sc,
            tc.tile_pool(name="ps_oc", bufs=3, space="PSUM") as ps_oc,
        ):
            # ---- persistent tiles ----
            # x^T lives in one [P, EC, T] tile loaded by two strided DMAs
            # (kproj-half0 token range first): 2 descriptor pushes instead
            # of 32, so the scalar queue is free for the rope swaps early on
            xts_all = persist.tile([P, EC, T], BF16, tag="xt", name="xt")
            xts = [xts_all[:, ec, :] for ec in range(EC)]

            cosq = persist.tile([P, S], BF16, tag="cosq")
            sinq = persist.tile([P, S], BF16, tag="sinq")
            cosk = persist.tile([P, T], BF16, tag="cosk")
            sink = persist.tile([P, T], BF16, tag="sink")
            masks = persist.tile([P, 2, 256], BF16, tag="masks")
            ones_full = persist.tile([P, P], BF16, tag="ones_full")
            bout = persist.tile([P, EMBED], BF16, tag="bout")

            # per-head normalized attention output (separate tiles so the
            # out projection's first matmuls wait only head 0's normalize)
            out_norm = [
                persist.tile([P, S], BF16, tag=f"onorm{h}", name=f"onorm{h}")
                for h in range(HEADS)
            ]

            # ---- weight DMA helpers (prefetched one step early) ----
            def load_wq(h, eng=None):
                # head 0 (startup) arrives in quarters so qproj's first
                # accumulation isn't gated on the full 512KB; prefetched
                # heads load in one push (fewer sync instrs + sem events)
                eng = eng or nc.sync
                wq_sb = wbig.tile([P, EC, HD], BF16, tag="wbig", name=f"wq{h}")
                if h == 0:
                    for i4 in range(4):
                        eng.dma_start(
                            wq_sb[:, 4 * i4 : 4 * i4 + 4, :],
                            WQ.ap()[h, :, 4 * i4 : 4 * i4 + 4, :],
                        )
                else:
                    eng.dma_start(wq_sb, WQ.ap()[h])
                return wq_sb

            def load_wk(h, eng=None):
                # head 0: four separate tiles (kproj's first matmuls wait
                # only the first quarter); prefetched heads: one tile, one
                # push (they have a full step of DMA lead time)
                eng = eng or nc.sync
                if h == 0:
                    wk_ts = []
                    for i4 in range(4):
                        wkq = wbig4.tile([P, 4, HD], BF16, tag="wbig4",
                                         name=f"wk{h}_{i4}")
                        eng.dma_start(
                            wkq, WK.ap()[h, :, 4 * i4 : 4 * i4 + 4, :]
                        )
                        wk_ts.append(wkq)
                    return wk_ts
                wk_sb = wbig.tile([P, EC, HD], BF16, tag="wbig", name=f"wk{h}")
                eng.dma_start(wk_sb, WK.ap()[h])
                return [wk_sb[:, 4 * i4 : 4 * i4 + 4, :] for i4 in range(4)]

            def load_wv(g, eng=None):
                eng = eng or nc.sync
                wv_qs = []
                for q in range(4):
                    wvq = wbig.tile([P, 4, 512], BF16, tag="wbig", name=f"wv{g}_{q}")
                    eng.dma_start(wvq, WV.ap()[g, :, 4 * q : 4 * q + 4, :])
                    wv_qs.append(wvq)
                return wv_qs

            def load_wo(eo, eng=None):
                eng = eng or nc.sync
                wo_qs = []
                for q in range(4):
                    woq = wbig.tile([P, 4, 512], BF16, tag="wbig", name=f"wo{eo}_{q}")
                    eng.dma_start(woq, WO.ap()[eo, :, 4 * q : 4 * q + 4, :])
                    wo_qs.append(woq)
                return wo_qs

            # ---- compute pieces ----
            def emit_qproj_mm(h, wq_sb, ec_lo, ec_hi, psq=None):
                if psq is None:
                    psq = ps_qv.tile([P, S], F32, tag="psqv", name=f"psq{h}")
                for ec in range(ec_lo, ec_hi):
                    nc.tensor.matmul(
                        psq,
                        wq_sb[:, ec, :],
                        xts[ec][:, WINDOW:T],
                        start=(ec == 0),
                        stop=(ec == EC - 1),
                    )
                return psq

            def emit_qrope(h, psq):
                """RoPE via straight-partition sin/cos muls on DVE (PSUM in),
                then a partition-half swap of the sin part (SBUF->SBUF DMA,
                one push on scalar + one on sync so they run in parallel).
                The final add runs on GPSIMD so the DMA latency never blocks
                the DVE queue (masks/norms).  SINQ is pre-swapped host-side."""
                q_sb = rope.tile([P, S], BF16, tag="qrope", name=f"q{h}")
                nc.vector.tensor_mul(q_sb, psq, cosq)
                qsin = rope.tile([P, S], BF16, tag="qsin", name=f"qsin{h}")
                nc.vector.tensor_mul(qsin, psq, sinq)
                qsw = rope.tile([P, S], BF16, tag="qsw", name=f"qsw{h}")
                nc.scalar.dma_start(qsw[0:64, :], qsin[64:128, :])
                nc.scalar.dma_start(qsw[64:128, :], qsin[0:64, :])
                return q_sb, qsw

            def emit_qrope_add(qctx):
                q_sb, qsw = qctx
                nc.vector.tensor_add(q_sb, q_sb, qsw)

            def emit_kproj_mm(h, wk_sb, half, kctx):
                lo, hi = (0, HT1) if half == 0 else (HT1, T)
                # both halves share one rotating bank: half1's matmuls
                # wait only half0's two DVE rope muls (early in the step),
                # freeing a PSUM bank for a third PV accumulator
                psk = ps_k.tile([P, hi - lo], F32, tag="psk",
                                name=f"psk{half}_{h}")
                for ec in range(EC):
                    nc.tensor.matmul(
                        psk,
                        wk_sb[ec // 4][:, ec % 4, :],
                        xts[ec][:, lo:hi],
                        start=(ec == 0),
                        stop=(ec == EC - 1),
                    )
                kctx[half] = psk

            def emit_krope_half(h, half, kctx):
                """RoPE for k tokens [lo:hi] except the final add.  Separate
                tiles per half so scores only wait the used half's writers."""
                lo, hi = (0, HT1) if half == 0 else (HT1, T)
                w = hi - lo
                psk = kctx[half]
                k_sb = rope.tile([P, w], BF16, tag=f"krope{half}",
                                 name=f"k{half}_{h}")
                ksin = rope.tile([P, w], BF16, tag=f"ksin{half}",
                                 name=f"ksin{half}_{h}")
                ksw = rope.tile([P, w], BF16, tag=f"ksw{half}",
                                name=f"ksw{half}_{h}")
                kctx[f"k{half}"] = k_sb
                kctx[f"ksw{half}"] = ksw
                nc.vector.tensor_mul(k_sb, psk, cosk[:, lo:hi])
                nc.vector.tensor_mul(ksin, psk, sink[:, lo:hi])
                nc.scalar.dma_start(ksw[0:64, :], ksin[64:128, :])
                nc.scalar.dma_start(ksw[64:128, :], ksin[0:64, :])

            def emit_krope_add(kctx, half):
                k_sb, ksw = kctx[f"k{half}"], kctx[f"ksw{half}"]
                nc.vector.tensor_add(k_sb, k_sb, ksw)

            def emit_vproj(g, tts):
                wv_qs = vctx_w[g]
                v_tiles = v_groups.setdefault(g, {})
                for tt in tts:
                    psv = ps_qv.tile([P, 512], F32, tag="psqv", name=f"psv{g}_{tt}")
                    for ec in range(EC):
                        nc.tensor.matmul(
                            psv,
                            xts[ec][:, tt * P : (tt + 1) * P],
                            wv_qs[ec // 4][:, ec % 4, :],
                            start=(ec == 0),
                            stop=(ec == EC - 1),
                        )
                    v_sb = vsb_pool.tile([P, 512], BF16, tag="vsb", name=f"v{g}_{tt}")
                    nc.scalar.copy(v_sb, psv)
                    v_tiles[tt] = v_sb

            def kslice(kctx, kc):
                # key-tile block kc (128 tokens) out of the two rope'd halves
                if kc < 3:
                    return kctx["k0"][:, kc * P : (kc + 1) * P]
                return kctx["k1"][:, (kc - 3) * P : (kc - 2) * P]

            def emit_scores(h, qc, q_sb, kctx):
                """[128,256] psc = [scores^T of k-block qc | k-block qc+1]
                for the 128 queries of chunk qc; exp + 0/1 mask -> et."""
                qs = qc * P
                psc = ps_sc.tile([P, 256], F32, tag="sc", name=f"sc{h}_{qc}")
                nc.tensor.matmul(
                    psc[:, 0:P], kslice(kctx, qc), q_sb[:, qs : qs + P],
                    start=True, stop=True,
                )
                nc.tensor.matmul(
                    psc[:, P : 2 * P], kslice(kctx, qc + 1), q_sb[:, qs : qs + P],
                    start=True, stop=True,
                )
                et = attn_pool.tile([P, 256], BF16, tag="attn",
                                    name=f"et{h}_{qc}")
                nc.scalar.activation(et, psc, mybir.ActivationFunctionType.Exp)
                nc.vector.tensor_mul(et, et, masks[:, 0 if qc == 0 else 1, :])
                return et

            def emit_pv(h, qc, et):
                hh = h % GH
                v_tiles = v_groups[h // GH]
                poc = ps_oc.tile([P, 256], F32, tag="oc", name=f"poc{h}_{qc}")
                nc.tensor.matmul(
                    poc[:, 0:P],
                    v_tiles[qc][:, hh * HD : (hh + 1) * HD],
                    et[:, 0:P],
                    start=True, stop=False,
                )
                nc.tensor.matmul(
                    poc[:, 0:P],
                    v_tiles[qc + 1][:, hh * HD : (hh + 1) * HD],
                    et[:, P : 2 * P],
                    start=False, stop=True,
                )
                nc.tensor.matmul(
                    poc[:, P : 2 * P], ones_full, et[:, 0:P],
                    start=True, stop=False,
                )
                nc.tensor.matmul(
                    poc[:, P : 2 * P], ones_full, et[:, P : 2 * P],
                    start=False, stop=True,
                )
                return poc

            def emit_normalize(h, qc, poc):
                # 1/rowsum as exp(-ln(rowsum)) on ACT: keeps the expensive
                # RECIPROCAL off the DVE queue
                qs = qc * P
                lns = small.tile([P, P], F32, tag="lns", name=f"ln{h}_{qc}")
                recip = small.tile([P, P], F32, tag="recip", name=f"rc{h}_{qc}")
                nc.scalar.activation(lns, poc[:, P : 2 * P],
                                     mybir.ActivationFunctionType.Ln)
                nc.scalar.activation(recip, lns,
                                     mybir.ActivationFunctionType.Exp,
                                     scale=-1.0)
                nc.vector.tensor_mul(
                    out_norm[h][:, qs : qs + P], poc[:, 0:P], recip
                )

            # ---- startup DMA: spread descriptor pushes over 3 queues ----
            v_groups = {}
            vctx_w = {}
            wk_next = load_wk(0, eng=nc.sync)       # sync: first weights
            wq_next = load_wq(0, eng=nc.sync)
            # scalar: x^T, kproj-half0 token range first so the first
            # accumulation group isn't gated on the full 640-token rows
            for ec in range(EC):
                nc.scalar.dma_start(xts_all[:, ec, 0:HT1], XT.ap()[:, ec, 0:HT1])
            for ec in range(EC):
                nc.scalar.dma_start(xts_all[:, ec, HT1:T], XT.ap()[:, ec, HT1:T])
            # gpsimd: rope tables (needed mid-step-0), V weights, the rest
            nc.gpsimd.dma_start(cosk, COSK.ap())
            nc.gpsimd.dma_start(sink, SINK.ap())
            nc.gpsimd.dma_start(cosq, COSQ.ap())
            nc.gpsimd.dma_start(sinq, SINQ.ap())
            vctx_w[0] = load_wv(0, eng=nc.gpsimd)
            nc.gpsimd.dma_start(masks, MASKS.ap().rearrange("m p q -> p m q"))
            nc.gpsimd.dma_start(ones_full, ONES.ap())

            # vproj spread for the 2-step-skew pipeline: group g tiles are
            # first consumed at attend(4g) = step 4g+2; weights for g load
            # at step 4g-1.  Group 0 starts at step 1 so its 2MB weight DMA
            # stays off the startup critical path.
            vproj_sched = {1: {0: [0, 1, 2]}, 2: {0: [3, 4]}}
            for g in range(1, GROUPS):
                vproj_sched.setdefault(4 * g, {})[g] = [0, 1]
                vproj_sched.setdefault(4 * g + 1, {})[g] = [2, 3]
                vproj_sched.setdefault(4 * g + 2, {})[g] = [4]

            # ---- software-pipelined main loop ----
            # step s: attend head s-2 (if any), project head s (if s < 16).
            # The 2-step skew keeps the startup window (first ~30us) free of
            # the V-weight stream and gives every RoPE chain >1 step of slack.
            hist = {}            # step -> (q_sb, kctx)
            for s in range(HEADS + 2):
                proj = s < HEADS
                prev = (s - 2, *hist.pop(s - 2)) if s >= 2 else None
                wq_sb, wk_sb = (wq_next, wk_next) if proj else (None, None)
                if proj:
                    kctx = {}
                    # prefetch next step's weights behind this step's compute
                    if s + 1 < HEADS:
                        wq_next = load_wq(s + 1)
                        wk_next = load_wk(s + 1)
                    elif s + 1 == HEADS:
                        wo_first = load_wo(0)
                        nc.sync.dma_start(bout, BOUT.ap())
                    if s % GH == 3 and s // GH + 1 < GROUPS:
                        vctx_w[s // GH + 1] = load_wv(s // GH + 1,
                                                      eng=nc.gpsimd)

                if proj:
                    emit_kproj_mm(s, wk_sb, 0, kctx)
                if prev is not None:
                    ph, pq, pk = prev
                    et0 = emit_scores(ph, 0, pq, pk)
                    et1 = emit_scores(ph, 1, pq, pk)
                if proj:
                    emit_krope_half(s, 0, kctx)
                    psq = emit_qproj_mm(s, wq_sb, 0, 8)
                if prev is not None:
                    poc0 = emit_pv(ph, 0, et0)
                    poc1 = emit_pv(ph, 1, et1)
                if proj:
                    emit_qproj_mm(s, wq_sb, 8, EC, psq)
                    qctx = emit_qrope(s, psq)
                if prev is not None:
                    et2 = emit_scores(ph, 2, pq, pk)
                    et3 = emit_scores(ph, 3, pq, pk)
                if proj:
                    emit_krope_add(kctx, 0)
                    emit_kproj_mm(s, wk_sb, 1, kctx)
                    emit_krope_half(s, 1, kctx)
                    emit_qrope_add(qctx)
                    emit_krope_add(kctx, 1)
                if proj and s in vproj_sched:
                    for g, tts in vproj_sched[s].items():
                        emit_vproj(g, tts)
                if prev is not None:
                    emit_normalize(ph, 0, poc0)
                    emit_normalize(ph, 1, poc1)
                    poc2 = emit_pv(ph, 2, et2)
                    poc3 = emit_pv(ph, 3, et3)
                    emit_normalize(ph, 2, poc2)
                    emit_normalize(ph, 3, poc3)
                if proj:
                    hist[s] = (qctx[0], kctx)

            # ---- out projection: OUT[t, e] = sum_hd out_norm^T . WO + bias ----
            wo_qs = wo_first
            for eo in range(4):
                e0 = eo * 512
                if eo > 0:
                    wo_qs = wo_next
                if eo + 1 < 4:
                    wo_next = load_wo(eo + 1)
                for tt in range(4):
                    pso = ps_qv.tile([P, 512], F32, tag="psqv", name=f"pso{eo}_{tt}")
                    for hd in range(HEADS):
                        nc.tensor.matmul(
                            pso,
                            out_norm[hd][:, tt * P : (tt + 1) * P],
                            wo_qs[hd // 4][:, hd % 4, :],
                            start=(hd == 0),
                            stop=(hd == HEADS - 1),
                        )
                    o_sb = outsb.tile([P, 512], BF16, tag="osb",
                                      name=f"osb{eo}_{tt}")
                    idx = eo * 4 + tt
                    # bias folded into the PSUM->SBUF move (GPSIMD cannot
                    # read PSUM); alternate DMA queues so the final tiles'
                    # stores drain in parallel
                    nc.vector.tensor_add(o_sb, pso, bout[:, e0 : e0 + 512])
                    dma_eng = nc.scalar if idx % 2 == 0 else nc.sync
                    dma_eng.dma_start(
                        OUT.ap()[tt * P : (tt + 1) * P, e0 : e0 + 512], o_sb
                    )

    if legalize:
        _legalize_single_wait(nc)
    return nc


def _rope_tables(pos, scale):
    """Feature-major [128, len(pos)] cos / sin'' tables in de-interleaved d order.

    cos'[i, t] = cos(pos_t * invf[i % 64]).  sin'' is PRE-SWAPPED for the
    straight-mul-then-swap scheme: the kernel computes s[d] = raw[d]*sin''[d]
    and then swaps partition halves, so sin''[0:64] = +sin (lands on the odd
    half after the swap) and sin''[64:128] = -sin.
    """
    inv_freq = 1.0 / (THETA ** (np.arange(0, HD, 2, dtype=np.float64) / HD))  # [64]
    ang = pos[None, :] * inv_freq[:, None]  # [64, T]
    cos = np.cos(ang)
    sin = np.sin(ang)
    cos_t = np.concatenate([cos, cos], axis=0) * scale
    sin_t = np.concatenate([sin, -sin], axis=0) * scale
    return cos_t.astype(np.float32), sin_t.astype(np.float32)


def _band_masks(start):
    """[2, 128, 256] multiplicative masks for 128-query chunks.

    Column block 0 = k-block kc==qc (valid iff kp >= r, and kc>0 resp.
    global key >= 0), block 1 = k-block kc==qc+1 (valid iff kp <= r).
    masks[0] is for qc==0 (whose diag block is the halo: all-invalid when
    start==0), masks[1] for qc>0.
    """
    kp = np.arange(P)[:, None]
    rq = np.arange(P)[None, :]
    A = (kp >= rq).astype(np.float32)
    Bm = (kp <= rq).astype(np.float32)
    A0 = np.zeros((P, P), np.float32) if start == 0 else A
    m_q0 = np.concatenate([A0, Bm], axis=1)
    m_rest = np.concatenate([A, Bm], axis=1)
    return np.stack([m_q0, m_rest])


_CACHED = {}
LAST_RESULT = {}
BF = ml_dtypes.bfloat16


def prepare_in_maps(x, W_qkv, W_out, b_out):
    x = np.asarray(x, dtype=np.float32)
    W_qkv = np.asarray(W_qkv, dtype=np.float32)
    W_out = np.asarray(W_out, dtype=np.float32)
    b_out = np.asarray(b_out, dtype=np.float32)

    # host-side weight layout prep
    perm = np.concatenate([np.arange(0, HD, 2), np.arange(1, HD, 2)])  # de-interleave
    w4 = W_qkv.reshape(EMBED, HEADS, HD, 3)
    # [h, e, d] -> [h, p, ec, d] partition-major contiguous
    WQa = w4[..., 0].transpose(1, 0, 2)[:, :, perm].reshape(HEADS, EMBED // P, P, HD)
    WQa = np.ascontiguousarray(WQa.transpose(0, 2, 1, 3)).astype(BF)
    WKa = w4[..., 1].transpose(1, 0, 2)[:, :, perm].reshape(HEADS, EMBED // P, P, HD)
    WKa = np.ascontiguousarray(WKa.transpose(0, 2, 1, 3)).astype(BF)
    # [e, f] -> [g, p, ec, 512]
    WVa = w4[..., 2].reshape(EMBED // P, P, 4, 512)
    WVa = np.ascontiguousarray(WVa.transpose(2, 1, 0, 3)).astype(BF)
    WOa = W_out.reshape(EMBED // P, P, 4, 512)
    WOa = np.ascontiguousarray(WOa.transpose(2, 1, 0, 3)).astype(BF)
    BO = np.ascontiguousarray(np.tile(b_out.reshape(1, EMBED), (P, 1))).astype(BF)

    in_maps = []
    for core in range(NCORES):
        b = core // 4
        start = (core % 4) * S
        # x^T with halo, zero-padded at the left for chunk 0
        xt = np.zeros((EMBED, T), dtype=np.float32)
        lo = start - WINDOW
        src = x[b, max(lo, 0) : start + S, :]  # [<=640, e]
        xt[:, T - src.shape[0] :] = src.T
        xt = np.ascontiguousarray(xt.reshape(EMBED // P, P, T).transpose(1, 0, 2))
        # rope tables: query positions start..start+512, key positions lo..start+512
        qpos = np.arange(start, start + S, dtype=np.float64)
        kpos = np.maximum(np.arange(lo, start + S, dtype=np.float64), 0.0)
        scale = 1.0 / math.sqrt(HD)
        cq, sq = _rope_tables(qpos, scale)
        ck, sk = _rope_tables(kpos, 1.0)
        in_maps.append(
            {
                "XT": xt.astype(BF),
                "WQ": WQa,
                "WK": WKa,
                "WV": WVa,
                "WO": WOa,
                "BOUT": BO,
                "COSQ": cq.astype(BF),
                "SINQ": sq.astype(BF),
                "COSK": ck.astype(BF),
                "SINK": sk.astype(BF),
                "MASKS": _band_masks(start).astype(BF),
                "ONES": np.ones((P, P), dtype=np.float32).astype(BF),
            }
        )
    return in_maps


def kernel(x, W_qkv, W_out, b_out):
    in_maps = prepare_in_maps(x, W_qkv, W_out, b_out)

    if "nc" not in _CACHED:
        _CACHED["nc"] = build_bass()
    nc = _CACHED["nc"]

    res = run_bass_kernel_spmd(nc, in_maps, core_ids=list(range(NCORES)))
    LAST_RESULT["res"] = res

    out = np.empty((B, L, EMBED), dtype=np.float32)
    for core in range(NCORES):
        b = core // 4
        start = (core % 4) * S
        out[b, start : start + S, :] = res.results[core]["OUT"].astype(np.float32)
    return out


# revision 39
# speedup vs baseline: 1.0149x; 1.0149x over previous
"""Sliding-window (banded) multi-head self-attention on 8 trn2 NeuronCores.

Sequence-parallel sharding: batch b, 2048 tokens -> 4 chunks of 512 queries;
core c handles batch c//4, chunk c%4.  Each core receives x^T for its 512
tokens plus a 128-token halo (zero-padded for chunk 0), computes
qkv projection + RoPE + banded attention (window 129) + out projection for
its rows, and returns [512, 2048].  No cross-core communication.

All matmul operands are bf16 (psum fp32): same 1 row/cycle PE rate as fp32r
but half the DMA / LDWEIGHTS traffic.  End-to-end numeric error vs the fp32
reference is ~5e-3, inside the 2e-2 gate.

Layout choices (all matmuls contract over the partition dim):
  - x^T resident in SBUF as [128, 16(e-chunk), 640(tok)]
  - Q^T/K^T per head feature-major [128(d), tok] straight out of PSUM;
    RoPE pairs de-interleaved host-side (d' = evens then odds) so
    rotate_half is a partition-half swap (SBUF->SBUF DMA).
  - V token-major [128(tok), d] (natural for PV lhsT).
  - attention in 128-query chunks: per (h, qc) one [128,256] PSUM tile
    holds scores^T for k-blocks qc (diag, upper-tri mask) and qc+1
    (lower-tri mask) side by side; one exp on ACT; one 0/1 mask multiply;
    2 PV + 2 replicated-ones rowsum matmuls accumulate into one poc bank;
    normalize with exp(-ln(rowsum)) on ACT.  Per query only 256 keys are
    scored (vs 384 with 256-query chunks) - 2/3 the attention PE work.
  - out projection accumulates 16 hd-chunks into per-head out_norm tiles
    (so its first matmuls wait only head 0's normalize); bias added on DVE;
    output stored bf16 (host casts back to fp32).
  - RoPE rotate-half: cos and (pre-swapped-table) sin muls both run on DVE
    straight from PSUM, the partition-half swap is an SBUF->SBUF DMA on the
    scalar queue, and the final add is back on DVE - no ACT copy in the
    chain, and no DMA-latency-coupled op ahead of the attention masks in
    the DVE stream.

Software pipeline with a 2-step skew (step s: project head s, attend head
s-2): every RoPE chain gets >1 step of slack, and the V-projection group 0
moves off the startup critical path (the first ~30us are DMA-bandwidth
bound).  Emission order keeps a couple microseconds of projection matmuls
in front of every dependent attention chain (exp->mask->PV, poc
recycling).  Startup DMA descriptor pushes are spread over the
sync/scalar/gpsimd queues.  Do NOT reorder the attend blocks or boost
their scheduler priority: the Tile scheduler is dataflow-driven and both
experiments regressed by 7-16us (measured).
"""

import math
import numpy as np
import ml_dtypes

import concourse.bass as bass
import concourse.tile as tile
from concourse import mybir
from concourse.bass_utils import run_bass_kernel_spmd
from concourse.vector_clock import ScopedClock, VectorClock


def _legalize_single_wait(nc):
    """This walrus build accepts only ONE sync-wait per lowered command
    ("Too many sync wait commands").  Move all but the last wait of every
    instruction onto single-wait NoOps prepended on the same engine: engines
    are in-order, so stalling on the NoOps is equivalent.  SP-issued DMAs are
    gated the same way (descriptor push happens in SP program order)."""
    nid = [0]
    for f in nc.m.functions:
        for blk in f.blocks:
            out = []
            changed = False
            for inst in blk.instructions:
                si = inst.sync_info
                waits = list(si.on_wait) if si and si.on_wait else []
                if len(waits) > 1:
                    changed = True
                    for w in waits[:-1]:
                        nop = mybir.InstNoOp(name=f"waitnop-{nid[0]}", ins=[], outs=[])
                        nid[0] += 1
                        nop.engine = inst.engine
                        nop.sync_info = mybir.SyncInfo(on_wait=[w], on_update=[])
                        out.append(nop)
                    inst.sync_info = mybir.SyncInfo(
                        on_wait=[waits[-1]], on_update=list(si.on_update or [])
                    )
                out.append(inst)
            if changed:
                blk.instructions = out
    return nc


def _install_drain_split_patch():
    """Split TileContext's closing drain into single-wait drains: walrus's
    CTRL_NO command rejects the catch-all drain ("Too many sync waits")."""
    if getattr(tile.TileContext, "_drain_split_patched", False):
        return

    def _patched(self, tick_clock, wait_clock):
        gvc = tick_clock.global_clock  # VectorClock over the 27 procs
        n = len(gvc)
        procs = [i for i in range(n) if gvc[i] > 0]
        for pi in procs:
            vc = VectorClock([gvc[i] if i == pi else 0 for i in range(n)])
            d = self.nc.sync.drain()
            wait_clock.add_sem_waits(d.ins, ScopedClock({None: vc}))
        self.nc.all_engine_barrier()
        assert self.sems is not None
        popped = self.nc._tile_sem_poison_stack.pop()
        assert popped is self._sem_poison
        self.nc.clear_and_free_semaphores(list(self.sems.allocated().values()))
        self.nc.all_engine_barrier()

    tile.TileContext._drain_and_barrier = _patched
    tile.TileContext._drain_split_patched = True


_install_drain_split_patch()

EMBED = 2048
HEADS = 16
HD = 128
WINDOW = 128
THETA = 10000.0
B = 2
L = 2048
S = 512            # queries per core
T = S + WINDOW     # k/v tokens per core (incl halo)
NCORES = 8
P = 128
F32 = mybir.dt.float32
BF16 = mybir.dt.bfloat16
HT1 = 384          # K-projection first token half (chunk-aligned)


def build_bass(legalize=True):
    nc = bass.Bass("TRN2", target_bir_lowering=False, debug=False)

    XT = nc.dram_tensor("XT", [P, EMBED // P, T], BF16, kind="ExternalInput")
    WQ = nc.dram_tensor("WQ", [HEADS, P, EMBED // P, HD], BF16, kind="ExternalInput")
    WK = nc.dram_tensor("WK", [HEADS, P, EMBED // P, HD], BF16, kind="ExternalInput")
    WV = nc.dram_tensor("WV", [4, P, EMBED // P, 512], BF16, kind="ExternalInput")
    WO = nc.dram_tensor("WO", [4, P, EMBED // P, 512], BF16, kind="ExternalInput")
    BOUT = nc.dram_tensor("BOUT", [P, EMBED], BF16, kind="ExternalInput")
    COSQ = nc.dram_tensor("COSQ", [P, S], BF16, kind="ExternalInput")
    SINQ = nc.dram_tensor("SINQ", [P, S], BF16, kind="ExternalInput")
    COSK = nc.dram_tensor("COSK", [P, T], BF16, kind="ExternalInput")
    SINK = nc.dram_tensor("SINK", [P, T], BF16, kind="ExternalInput")
    MASKS = nc.dram_tensor("MASKS", [2, P, 256], BF16, kind="ExternalInput")
    ONES = nc.dram_tensor("ONES", [P, P], BF16, kind="ExternalInput")
    OUT = nc.dram_tensor("OUT", [S, EMBED], BF16, kind="ExternalOutput")

    EC = EMBED // P  # 16 e-chunks
    GROUPS = 4       # head groups of 4 (for V projection at N=512)
    GH = HEADS // GROUPS

    with tile.TileContext(nc) as tc:
        with (
            tc.tile_pool(name="persist", bufs=1) as persist,
            tc.tile_pool(name="wbig", bufs=8) as wbig,
            tc.tile_pool(name="wbig4", bufs=8) as wbig4,
            tc.tile_pool(name="rope", bufs=3) as rope,
            tc.tile_pool(name="vsb", bufs=12) as vsb_pool,
            tc.tile_pool(name="attn", bufs=8) as attn_pool,
            tc.tile_pool(name="small", bufs=4) as small,
            tc.tile_pool(name="outsb", bufs=4) as outsb,
            tc.tile_pool(name="ps_qv", bufs=2, space="PSUM") as ps_qv,
            tc.tile_pool(name="ps_k", bufs=1, space="PSUM") as ps_k,
            tc.tile_pool(name="ps_sc", bufs=2, space="PSUM") as ps_sc,
            tc.tile_pool(name="ps_oc", bufs=3, space="PSUM") as ps_oc,
        ):
            # ---- persistent tiles ----
            # x^T lives in one [P, EC, T] tile loaded by two strided DMAs
            # (kproj-half0 token range first): 2 descriptor pushes instead
            # of 32, so the scalar queue is free for the rope swaps early on
            xts_all = persist.tile([P, EC, T], BF16, tag="xt", name="xt")
            xts = [xts_all[:, ec, :] for ec in range(EC)]

            cosq = persist.tile([P, S], BF16, tag="cosq")
            sinq = persist.tile([P, S], BF16, tag="sinq")
            cosk = persist.tile([P, T], BF16, tag="cosk")
            sink = persist.tile([P, T], BF16, tag="sink")
            masks = persist.tile([P, 2, 256], BF16, tag="masks")
            ones_full = persist.tile([P, P], BF16, tag="ones_full")
            bout = persist.tile([P, EMBED], BF16, tag="bout")

            # per-head normalized attention output (separate tiles so the
            # out projection's first matmuls wait only head 0's normalize)
            out_norm = [
                persist.tile([P, S], BF16, tag=f"onorm{h}", name=f"onorm{h}")
                for h in range(HEADS)
            ]

            # ---- weight DMA helpers (prefetched one step early) ----
            def load_wq(h, eng=None):
                # head 0 (startup) arrives in quarters so qproj's first
                # accumulation isn't gated on the full 512KB; prefetched
                # heads load in one push (fewer sync instrs + sem events)
                eng = eng or nc.sync
                wq_sb = wbig.tile([P, EC, HD], BF16, tag="wbig", name=f"wq{h}")
                if h == 0:
                    for i4 in range(4):
                        eng.dma_start(
                            wq_sb[:, 4 * i4 : 4 * i4 + 4, :],
                            WQ.ap()[h, :, 4 * i4 : 4 * i4 + 4, :],
                        )
                else:
                    eng.dma_start(wq_sb, WQ.ap()[h])
                return wq_sb

            def load_wk(h, eng=None):
                # head 0: four separate tiles (kproj's first matmuls wait
                # only the first quarter); prefetched heads: one tile, one
                # push (they have a full step of DMA lead time)
                eng = eng or nc.sync
                if h == 0:
                    wk_ts = []
                    for i4 in range(4):
                        wkq = wbig4.tile([P, 4, HD], BF16, tag="wbig4",
                                         name=f"wk{h}_{i4}")
                        eng.dma_start(
                            wkq, WK.ap()[h, :, 4 * i4 : 4 * i4 + 4, :]
                        )
                        wk_ts.append(wkq)
                    return wk_ts
                wk_sb = wbig.tile([P, EC, HD], BF16, tag="wbig", name=f"wk{h}")
                eng.dma_start(wk_sb, WK.ap()[h])
                return [wk_sb[:, 4 * i4 : 4 * i4 + 4, :] for i4 in range(4)]

            def load_wv(g, eng=None):
                eng = eng or nc.sync
                wv_qs = []
                for q in range(4):
                    wvq = wbig.tile([P, 4, 512], BF16, tag="wbig", name=f"wv{g}_{q}")
                    eng.dma_start(
                        wvq[:, 0:2, :], WV.ap()[g, :, 4 * q : 4 * q + 2, :]
                    )
                    eng.dma_start(
                        wvq[:, 2:4, :], WV.ap()[g, :, 4 * q + 2 : 4 * q + 4, :]
                    )
                    wv_qs.append(wvq)
                return wv_qs

            def load_wo(eo, eng=None):
                eng = eng or nc.sync
                wo_qs = []
                for q in range(4):
                    woq = wbig.tile([P, 4, 512], BF16, tag="wbig", name=f"wo{eo}_{q}")
                    eng.dma_start(
                        woq[:, 0:2, :], WO.ap()[eo, :, 4 * q : 4 * q + 2, :]
                    )
                    eng.dma_start(
                        woq[:, 2:4, :], WO.ap()[eo, :, 4 * q + 2 : 4 * q + 4, :]
                    )
                    wo_qs.append(woq)
                return wo_qs

            # ---- compute pieces ----
            def emit_qproj_mm(h, wq_sb, ec_lo, ec_hi, psq=None):
                if psq is None:
                    psq = ps_qv.tile([P, S], F32, tag="psqv", name=f"psq{h}")
                for ec in range(ec_lo, ec_hi):
                    nc.tensor.matmul(
                        psq,
                        wq_sb[:, ec, :],
                        xts[ec][:, WINDOW:T],
                        start=(ec == 0),
                        stop=(ec == EC - 1),
                    )
                return psq

            def emit_qrope(h, psq):
                """RoPE via straight-partition sin/cos muls on DVE (PSUM in),
                then a partition-half swap of the sin part (SBUF->SBUF DMA,
                one push on scalar + one on sync so they run in parallel).
                The final add runs on GPSIMD so the DMA latency never blocks
                the DVE queue (masks/norms).  SINQ is pre-swapped host-side."""
                q_sb = rope.tile([P, S], BF16, tag="qrope", name=f"q{h}")
                nc.vector.tensor_mul(q_sb, psq, cosq)
                qsin = rope.tile([P, S], BF16, tag="qsin", name=f"qsin{h}")
                nc.vector.tensor_mul(qsin, psq, sinq)
                qsw = rope.tile([P, S], BF16, tag="qsw", name=f"qsw{h}")
                nc.scalar.dma_start(qsw[0:64, :], qsin[64:128, :])
                nc.scalar.dma_start(qsw[64:128, :], qsin[0:64, :])
                return q_sb, qsw

            def emit_qrope_add(qctx):
                q_sb, qsw = qctx
                nc.vector.tensor_add(q_sb, q_sb, qsw)

            def emit_kproj_mm(h, wk_sb, half, kctx):
                lo, hi = (0, HT1) if half == 0 else (HT1, T)
                # both halves share one rotating bank: half1's matmuls
                # wait only half0's two DVE rope muls (early in the step),
                # freeing a PSUM bank for a third PV accumulator
                psk = ps_k.tile([P, hi - lo], F32, tag="psk",
                                name=f"psk{half}_{h}")
                for ec in range(EC):
                    nc.tensor.matmul(
                        psk,
                        wk_sb[ec // 4][:, ec % 4, :],
                        xts[ec][:, lo:hi],
                        start=(ec == 0),
                        stop=(ec == EC - 1),
                    )
                kctx[half] = psk

            def emit_krope_half(h, half, kctx):
                """RoPE for k tokens [lo:hi] except the final add.  Separate
                tiles per half so scores only wait the used half's writers."""
                lo, hi = (0, HT1) if half == 0 else (HT1, T)
                w = hi - lo
                psk = kctx[half]
                k_sb = rope.tile([P, w], BF16, tag=f"krope{half}",
                                 name=f"k{half}_{h}")
                ksin = rope.tile([P, w], BF16, tag=f"ksin{half}",
                                 name=f"ksin{half}_{h}")
                ksw = rope.tile([P, w], BF16, tag=f"ksw{half}",
                                name=f"ksw{half}_{h}")
                kctx[f"k{half}"] = k_sb
                kctx[f"ksw{half}"] = ksw
                nc.vector.tensor_mul(k_sb, psk, cosk[:, lo:hi])
                nc.vector.tensor_mul(ksin, psk, sink[:, lo:hi])
                nc.scalar.dma_start(ksw[0:64, :], ksin[64:128, :])
                nc.scalar.dma_start(ksw[64:128, :], ksin[0:64, :])

            def emit_krope_add(kctx, half):
                k_sb, ksw = kctx[f"k{half}"], kctx[f"ksw{half}"]
                nc.vector.tensor_add(k_sb, k_sb, ksw)

            def emit_vproj(g, tts):
                wv_qs = vctx_w[g]
                v_tiles = v_groups.setdefault(g, {})
                for tt in tts:
                    psv = ps_qv.tile([P, 512], F32, tag="psqv", name=f"psv{g}_{tt}")
                    for ec in range(EC):
                        nc.tensor.matmul(
                            psv,
                            xts[ec][:, tt * P : (tt + 1) * P],
                            wv_qs[ec // 4][:, ec % 4, :],
                            start=(ec == 0),
                            stop=(ec == EC - 1),
                        )
                    v_sb = vsb_pool.tile([P, 512], BF16, tag="vsb", name=f"v{g}_{tt}")
                    nc.scalar.copy(v_sb, psv)
                    v_tiles[tt] = v_sb

            def kslice(kctx, kc):
                # key-tile block kc (128 tokens) out of the two rope'd halves
                if kc < 3:
                    return kctx["k0"][:, kc * P : (kc + 1) * P]
                return kctx["k1"][:, (kc - 3) * P : (kc - 2) * P]

            def emit_scores(h, qc, q_sb, kctx):
                """[128,256] psc = [scores^T of k-block qc | k-block qc+1]
                for the 128 queries of chunk qc; exp + 0/1 mask -> et."""
                qs = qc * P
                psc = ps_sc.tile([P, 256], F32, tag="sc", name=f"sc{h}_{qc}")
                nc.tensor.matmul(
                    psc[:, 0:P], kslice(kctx, qc), q_sb[:, qs : qs + P],
                    start=True, stop=True,
                )
                nc.tensor.matmul(
                    psc[:, P : 2 * P], kslice(kctx, qc + 1), q_sb[:, qs : qs + P],
                    start=True, stop=True,
                )
                et = attn_pool.tile([P, 256], BF16, tag="attn",
                                    name=f"et{h}_{qc}")
                nc.scalar.activation(et, psc, mybir.ActivationFunctionType.Exp)
                nc.vector.tensor_mul(et, et, masks[:, 0 if qc == 0 else 1, :])
                return et

            def emit_pv(h, qc, et):
                hh = h % GH
                v_tiles = v_groups[h // GH]
                poc = ps_oc.tile([P, 256], F32, tag="oc", name=f"poc{h}_{qc}")
                nc.tensor.matmul(
                    poc[:, 0:P],
                    v_tiles[qc][:, hh * HD : (hh + 1) * HD],
                    et[:, 0:P],
                    start=True, stop=False,
                )
                nc.tensor.matmul(
                    poc[:, 0:P],
                    v_tiles[qc + 1][:, hh * HD : (hh + 1) * HD],
                    et[:, P : 2 * P],
                    start=False, stop=True,
                )
                nc.tensor.matmul(
                    poc[:, P : 2 * P], ones_full, et[:, 0:P],
                    start=True, stop=False,
                )
                nc.tensor.matmul(
                    poc[:, P : 2 * P], ones_full, et[:, P : 2 * P],
                    start=False, stop=True,
                )
                return poc

            def emit_normalize(h, qc, poc):
                # 1/rowsum as exp(-ln(rowsum)) on ACT: keeps the expensive
                # RECIPROCAL off the DVE queue
                qs = qc * P
                lns = small.tile([P, P], F32, tag="lns", name=f"ln{h}_{qc}")
                recip = small.tile([P, P], F32, tag="recip", name=f"rc{h}_{qc}")
                nc.scalar.activation(lns, poc[:, P : 2 * P],
                                     mybir.ActivationFunctionType.Ln)
                nc.scalar.activation(recip, lns,
                                     mybir.ActivationFunctionType.Exp,
                                     scale=-1.0)
                nc.vector.tensor_mul(
                    out_norm[h][:, qs : qs + P], poc[:, 0:P], recip
                )

            # ---- startup DMA: spread descriptor pushes over 3 queues ----
            v_groups = {}
            vctx_w = {}
            wk_next = load_wk(0, eng=nc.sync)       # sync: first weights
            wq_next = load_wq(0, eng=nc.sync)
            # scalar: x^T, kproj-half0 token range first so the first
            # accumulation group isn't gated on the full 640-token rows
            for ec in range(EC):
                nc.scalar.dma_start(xts_all[:, ec, 0:HT1], XT.ap()[:, ec, 0:HT1])
            for ec in range(EC):
                nc.scalar.dma_start(xts_all[:, ec, HT1:T], XT.ap()[:, ec, HT1:T])
            # gpsimd: rope tables (needed mid-step-0), V weights, the rest
            nc.gpsimd.dma_start(cosk, COSK.ap())
            nc.gpsimd.dma_start(sink, SINK.ap())
            nc.gpsimd.dma_start(cosq, COSQ.ap())
            nc.gpsimd.dma_start(sinq, SINQ.ap())
            vctx_w[0] = load_wv(0, eng=nc.gpsimd)
            nc.gpsimd.dma_start(masks, MASKS.ap().rearrange("m p q -> p m q"))
            nc.gpsimd.dma_start(ones_full, ONES.ap())

            # vproj spread for the 2-step-skew pipeline: group g tiles are
            # first consumed at attend(4g) = step 4g+2; weights for g load
            # at step 4g-1.  Group 0 starts at step 1 so its 2MB weight DMA
            # stays off the startup critical path.
            vproj_sched = {1: {0: [0, 1, 2]}, 2: {0: [3, 4]}}
            for g in range(1, GROUPS):
                vproj_sched.setdefault(4 * g, {})[g] = [0, 1]
                vproj_sched.setdefault(4 * g + 1, {})[g] = [2, 3]
                vproj_sched.setdefault(4 * g + 2, {})[g] = [4]

            # ---- software-pipelined main loop ----
            # step s: attend head s-2 (if any), project head s (if s < 16).
            # The 2-step skew keeps the startup window (first ~30us) free of
            # the V-weight stream and gives every RoPE chain >1 step of slack.
            hist = {}            # step -> (q_sb, kctx)
            for s in range(HEADS + 2):
                proj = s < HEADS
                prev = (s - 2, *hist.pop(s - 2)) if s >= 2 else None
                wq_sb, wk_sb = (wq_next, wk_next) if proj else (None, None)
                if proj:
                    kctx = {}
                    # prefetch next step's weights behind this step's compute
                    if s + 1 < HEADS:
                        wq_next = load_wq(s + 1)
                        wk_next = load_wk(s + 1)
                    elif s + 1 == HEADS:
                        wo_first = load_wo(0)
                        nc.sync.dma_start(bout, BOUT.ap())
                    if s % GH == 3 and s // GH + 1 < GROUPS:
                        vctx_w[s // GH + 1] = load_wv(s // GH + 1,
                                                      eng=nc.gpsimd)

                if proj:
                    emit_kproj_mm(s, wk_sb, 0, kctx)
                if prev is not None:
                    ph, pq, pk = prev
                    et0 = emit_scores(ph, 0, pq, pk)
                    et1 = emit_scores(ph, 1, pq, pk)
                if proj:
                    emit_krope_half(s, 0, kctx)
                    psq = emit_qproj_mm(s, wq_sb, 0, 8)
                if prev is not None:
                    poc0 = emit_pv(ph, 0, et0)
                    poc1 = emit_pv(ph, 1, et1)
                if proj:
                    emit_qproj_mm(s, wq_sb, 8, EC, psq)
                    qctx = emit_qrope(s, psq)
                if prev is not None:
                    et2 = emit_scores(ph, 2, pq, pk)
                    et3 = emit_scores(ph, 3, pq, pk)
                if proj:
                    emit_krope_add(kctx, 0)
                    emit_kproj_mm(s, wk_sb, 1, kctx)
                    emit_krope_half(s, 1, kctx)
                    emit_qrope_add(qctx)
                    emit_krope_add(kctx, 1)
                if proj and s in vproj_sched:
                    for g, tts in vproj_sched[s].items():
                        emit_vproj(g, tts)
                if prev is not None:
                    emit_normalize(ph, 0, poc0)
                    emit_normalize(ph, 1, poc1)
                    poc2 = emit_pv(ph, 2, et2)
                    poc3 = emit_pv(ph, 3, et3)
                    emit_normalize(ph, 2, poc2)
                    emit_normalize(ph, 3, poc3)
                if proj:
                    hist[s] = (qctx[0], kctx)

            # ---- out projection: OUT[t, e] = sum_hd out_norm^T . WO + bias ----
            wo_qs = wo_first
            for eo in range(4):
                e0 = eo * 512
                if eo > 0:
                    wo_qs = wo_next
                if eo + 1 < 4:
                    wo_next = load_wo(eo + 1)
                for tt in range(4):
                    pso = ps_qv.tile([P, 512], F32, tag="psqv", name=f"pso{eo}_{tt}")
                    for hd in range(HEADS):
                        nc.tensor.matmul(
                            pso,
                            out_norm[hd][:, tt * P : (tt + 1) * P],
                            wo_qs[hd // 4][:, hd % 4, :],
                            start=(hd == 0),
                            stop=(hd == HEADS - 1),
                        )
                    o_sb = outsb.tile([P, 512], BF16, tag="osb",
                                      name=f"osb{eo}_{tt}")
                    idx = eo * 4 + tt
                    # bias folded into the PSUM->SBUF move (GPSIMD cannot
                    # read PSUM); alternate DMA queues so the final tiles'
                    # stores drain in parallel
                    nc.vector.tensor_add(o_sb, pso, bout[:, e0 : e0 + 512])
                    dma_eng = nc.scalar if idx % 2 == 0 else nc.sync
                    dma_eng.dma_start(
                        OUT.ap()[tt * P : (tt + 1) * P, e0 : e0 + 512], o_sb
                    )

    if legalize:
        _legalize_single_wait(nc)
    return nc


def _rope_tables(pos, scale):
    """Feature-major [128, len(pos)] cos / sin'' tables in de-interleaved d order.

    cos'[i, t] = cos(pos_t * invf[i % 64]).  sin'' is PRE-SWAPPED for the
    straight-mul-then-swap scheme: the kernel computes s[d] = raw[d]*sin''[d]
    and then swaps partition halves, so sin''[0:64] = +sin (lands on the odd
    half after the swap) and sin''[64:128] = -sin.
    """
    inv_freq = 1.0 / (THETA ** (np.arange(0, HD, 2, dtype=np.float64) / HD))  # [64]
    ang = pos[None, :] * inv_freq[:, None]  # [64, T]
    cos = np.cos(ang)
    sin = np.sin(ang)
    cos_t = np.concatenate([cos, cos], axis=0) * scale
    sin_t = np.concatenate([sin, -sin], axis=0) * scale
    return cos_t.astype(np.float32), sin_t.astype(np.float32)


def _band_masks(start):
    """[2, 128, 256] multiplicative masks for 128-query chunks.

    Column block 0 = k-block kc==qc (valid iff kp >= r, and kc>0 resp.
    global key >= 0), block 1 = k-block kc==qc+1 (valid iff kp <= r).
    masks[0] is for qc==0 (whose diag block is the halo: all-invalid when
    start==0), masks[1] for qc>0.
    """
    kp = np.arange(P)[:, None]
    rq = np.arange(P)[None, :]
    A = (kp >= rq).astype(np.float32)
    Bm = (kp <= rq).astype(np.float32)
    A0 = np.zeros((P, P), np.float32) if start == 0 else A
    m_q0 = np.concatenate([A0, Bm], axis=1)
    m_rest = np.concatenate([A, Bm], axis=1)
    return np.stack([m_q0, m_rest])


_CACHED = {}
LAST_RESULT = {}
BF = ml_dtypes.bfloat16


def prepare_in_maps(x, W_qkv, W_out, b_out):
    x = np.asarray(x, dtype=np.float32)
    W_qkv = np.asarray(W_qkv, dtype=np.float32)
    W_out = np.asarray(W_out, dtype=np.float32)
    b_out = np.asarray(b_out, dtype=np.float32)

    # host-side weight layout prep
    perm = np.concatenate([np.arange(0, HD, 2), np.arange(1, HD, 2)])  # de-interleave
    w4 = W_qkv.reshape(EMBED, HEADS, HD, 3)
    # [h, e, d] -> [h, p, ec, d] partition-major contiguous
    WQa = w4[..., 0].transpose(1, 0, 2)[:, :, perm].reshape(HEADS, EMBED // P, P, HD)
    WQa = np.ascontiguousarray(WQa.transpose(0, 2, 1, 3)).astype(BF)
    WKa = w4[..., 1].transpose(1, 0, 2)[:, :, perm].reshape(HEADS, EMBED // P, P, HD)
    WKa = np.ascontiguousarray(WKa.transpose(0, 2, 1, 3)).astype(BF)
    # [e, f] -> [g, p, ec, 512]
    WVa = w4[..., 2].reshape(EMBED // P, P, 4, 512)
    WVa = np.ascontiguousarray(WVa.transpose(2, 1, 0, 3)).astype(BF)
    WOa = W_out.reshape(EMBED // P, P, 4, 512)
    WOa = np.ascontiguousarray(WOa.transpose(2, 1, 0, 3)).astype(BF)
    BO = np.ascontiguousarray(np.tile(b_out.reshape(1, EMBED), (P, 1))).astype(BF)

    in_maps = []
    for core in range(NCORES):
        b = core // 4
        start = (core % 4) * S
        # x^T with halo, zero-padded at the left for chunk 0
        xt = np.zeros((EMBED, T), dtype=np.float32)
        lo = start - WINDOW
        src = x[b, max(lo, 0) : start + S, :]  # [<=640, e]
        xt[:, T - src.shape[0] :] = src.T
        xt = np.ascontiguousarray(xt.reshape(EMBED // P, P, T).transpose(1, 0, 2))
        # rope tables: query positions start..start+512, key positions lo..start+512
        qpos = np.arange(start, start + S, dtype=np.float64)
        kpos = np.maximum(np.arange(lo, start + S, dtype=np.float64), 0.0)
        scale = 1.0 / math.sqrt(HD)
        cq, sq = _rope_tables(qpos, scale)
        ck, sk = _rope_tables(kpos, 1.0)
        in_maps.append(
            {
                "XT": xt.astype(BF),
                "WQ": WQa,
                "WK": WKa,
                "WV": WVa,
                "WO": WOa,
                "BOUT": BO,
                "COSQ": cq.astype(BF),
                "SINQ": sq.astype(BF),
                "COSK": ck.astype(BF),
                "SINK": sk.astype(BF),
                "MASKS": _band_masks(start).astype(BF),
                "ONES": np.ones((P, P), dtype=np.float32).astype(BF),
            }
        )
    return in_maps


def kernel(x, W_qkv, W_out, b_out):
    in_maps = prepare_in_maps(x, W_qkv, W_out, b_out)

    if "nc" not in _CACHED:
        _CACHED["nc"] = build_bass()
    nc = _CACHED["nc"]

    res = run_bass_kernel_spmd(nc, in_maps, core_ids=list(range(NCORES)))
    LAST_RESULT["res"] = res

    out = np.empty((B, L, EMBED), dtype=np.float32)
    for core in range(NCORES):
        b = core // 4
        start = (core % 4) * S
        out[b, start : start + S, :] = res.results[core]["OUT"].astype(np.float32)
    return out
